# revision 2
# baseline (speedup 1.0000x reference)
"""Trainium2 Bass kernel for nn_Decoder (MusicVAE-style hierarchical decoder).

Strategy (8 NeuronCores, data-parallel over batch, no inter-core comms):
  - Conductor LSTM (16 sequential levels, batch 32/core) computes per-level
    embeddings.
  - Decoder levels are INDEPENDENT (initial state from dec_h0/dec_c0,
    note0=0), so all 16 levels are batched: effective decoder batch
    16*32 = 512 rows per core, 16 sequential note steps.
  - The conductor embedding is constant within a level, so its gate
    contribution (emb @ Wih[:, :H].T + bias) is precomputed once ("ge").
  - Everything lives feature-major: [features on partitions, rows free].
    Weights are the stationary matmul operand, activations stream.
  - fp8(e4m3) matmuls with DoubleRow perf mode (2 k-subtiles per MM) for
    all 512-row streams; conductor recurrent matmuls (32-row streams) are
    fp8 without DoubleRow (same speed as bf16, FWL weight loads).
    fp32 PSUM accumulation; gates bf16; c state fp32; note output bf16
    with an fp8 mirror feeding the next step's matmul.
"""
import numpy as np
import ml_dtypes

import concourse.bacc as bacc
import concourse.tile as tile
import concourse.mybir as mybir
from concourse.bass_utils import run_bass_kernel_spmd

bf16 = ml_dtypes.bfloat16
f8 = ml_dtypes.float8_e4m3
F32 = mybir.dt.float32
BF = mybir.dt.bfloat16
F8 = mybir.dt.float8e4
AF = mybir.ActivationFunctionType
DR = mybir.MatmulPerfMode.DoubleRow

NCORES = 8
B, Z, H, T = 256, 512, 1024, 512
L, NS = 16, 16
Bc = B // NCORES            # 32 batch rows per core
R = L * Bc                  # 512 decoder rows per core (levels x batch)
HK, TK, ZK = H // 128, T // 128, Z // 128   # 8, 4, 4
G = 4 * H // 128            # 32 gate chunks of 128


def _declare(nc):
    d = {}
    ei = dict(kind="ExternalInput")
    d["ones"] = nc.dram_tensor("ones", [1, R], BF, **ei)
    d["cbias"] = nc.dram_tensor("cbias", [1, 4 * H], BF, **ei)
    d["dbias"] = nc.dram_tensor("dbias", [1, 4 * H], BF, **ei)
    d["obias"] = nc.dram_tensor("obias", [128, TK], F32, **ei)
    d["zT"] = nc.dram_tensor("zT", [128, ZK, R], F8, **ei)
    d["h0T"] = nc.dram_tensor("h0T", [128, HK, R], F8, **ei)
    d["c0T"] = nc.dram_tensor("c0T", [128, HK, R], F32, **ei)
    d["cwih"] = nc.dram_tensor("cwih", [128, ZK, 4 * H], F8, **ei)
    d["cwhh"] = nc.dram_tensor("cwhh", [128, HK, 4 * H], F8, **ei)
    d["dwe"] = nc.dram_tensor("dwe", [G, 128, HK * 128], F8, **ei)
    d["dwn"] = nc.dram_tensor("dwn", [128, TK, 4 * H], F8, **ei)
    d["dwhh"] = nc.dram_tensor("dwhh", [128, HK, 4 * H], F8, **ei)
    d["owt"] = nc.dram_tensor("owt", [128, HK, T], F8, **ei)
    d["outbuf"] = nc.dram_tensor("outbuf", [NS, TK, 128, R], BF,
                                 kind="ExternalOutput")
    return d


def _mm_dr(nc, out, w3, x3, ks, ms, start, stop):
    """DoubleRow fp8 matmul over k-subtile pair (ks, ks+1).
    w3: [128, K, 4H-ish] weight tile; x3: [128, K, R] activation tile."""
    return nc.tensor.matmul(out, w3[:, ks:ks + 2, ms], x3[:, ks:ks + 2, :],
                            start=start, stop=stop, perf_mode=DR)


def _body(nc, tc, d):
    import contextlib
    with contextlib.ExitStack() as ctx:
        Pp = ctx.enter_context(tc.tile_pool(name="persist", bufs=1))

        t_ones = Pp.tile([1, R], BF, tag="ones")
        nc.sync.dma_start(t_ones[:], d["ones"][:])
        t_ob = Pp.tile([128, TK], F32, tag="obias")
        nc.sync.dma_start(t_ob[:], d["obias"][:])
        t_emb = Pp.tile([128, HK, R], F8, tag="emb")
        t_h = [Pp.tile([128, HK, R], F8, tag=f"hT{i}", name=f"hT{i}")
               for i in (0, 1)]
        t_c = Pp.tile([128, HK, R], F32, tag="c")
        t_note = Pp.tile([128, TK, R], BF, tag="note")
        t_note8 = Pp.tile([128, TK, R], F8, tag="note8")
        nc.gpsimd.dma_start(t_h[0][:], d["h0T"][:])
        nc.gpsimd.dma_start(t_c[:], d["c0T"][:])

        # ---------------- conductor ----------------
        with tc.tile_pool(name="cond", bufs=1) as Pc, \
             tc.tile_pool(name="ctmp", bufs=2) as Pt, \
             tc.tile_pool(name="cps", bufs=4, space="PSUM") as PSc, \
             tc.tile_pool(name="gzps", bufs=2, space="PSUM") as PSz:
            t_cb = Pc.tile([1, 4 * H], BF, tag="cbias")
            nc.sync.dma_start(t_cb[:], d["cbias"][:])
            t_cwih = Pc.tile([128, ZK, 4 * H], F8, tag="cwih")
            nc.sync.dma_start(t_cwih[:], d["cwih"][:])
            t_zT = Pc.tile([128, ZK, R], F8, tag="zT")
            nc.sync.dma_start(t_zT[:], d["zT"][:])
            t_cwhh = Pc.tile([128, HK, 4 * H], F8, tag="cwhh")
            nc.sync.dma_start(t_cwhh[:], d["cwhh"][:])
            t_gz = Pc.tile([128, G, R], BF, tag="gz")
            t_cc = Pc.tile([128, HK, Bc], F32, tag="cc")

            # gz = z @ cond_Wih.T + cond_b for all levels at once (fp8 DR)
            for m in range(G):
                ms = slice(m * 128, (m + 1) * 128)
                ps = PSz.tile([128, R], F32, tag="gzp")
                nc.tensor.matmul(ps[:], t_cb[0:1, ms], t_ones[:],
                                 start=True, stop=False)
                for k in range(0, ZK, 2):
                    _mm_dr(nc, ps[:], t_cwih, t_zT, k, ms,
                           False, (k == ZK - 2))
                nc.vector.tensor_copy(t_gz[:, m, :], ps[:])

            # sequential levels (fp8 non-DR: 32-row streams)
            for _crep in range(COND_REPS):
              for lv in range(L):
                  cs = slice(lv * Bc, (lv + 1) * Bc)
                  ps_prev = slice((lv - 1) * Bc, lv * Bc)
                  for p in range(HK):
                      mi, mf, mg, mo = p, HK + p, 2 * HK + p, 3 * HK + p
                      ti = Pt.tile([128, Bc], BF, tag="ti")
                      tg = Pt.tile([128, Bc], BF, tag="tg")
                      to = Pt.tile([128, Bc], BF, tag="to")
                      tcn = Pt.tile([128, Bc], BF, tag="tcn")
                      tm1 = Pt.tile([128, Bc], BF, tag="tm1")
                      if lv == 0:
                          # h0 == 0: gates are just gz; c0 == 0: c = sig(i)*tanh(g)
                          nc.scalar.activation(ti[:], t_gz[:, mi, cs], AF.Sigmoid)
                          nc.scalar.activation(tg[:], t_gz[:, mg, cs], AF.Tanh)
                          nc.scalar.activation(to[:], t_gz[:, mo, cs], AF.Sigmoid)
                          nc.vector.tensor_mul(t_cc[:, p, :], ti[:], tg[:])
                      else:
                          ps = PSc.tile([128, 4, Bc], F32, tag="cgp")
                          for gi, m in enumerate((mi, mf, mg, mo)):
                              ms = slice(m * 128, (m + 1) * 128)
                              for k in range(HK):
                                  nc.tensor.matmul(
                                      ps[:, gi, :], t_cwhh[:, k, ms],
                                      t_emb[:, k, ps_prev],
                                      start=(k == 0), stop=(k == HK - 1))
                          tf = Pt.tile([128, Bc], BF, tag="tf")
                          tm2 = Pt.tile([128, Bc], F32, tag="tm2")
                          gsi = Pt.tile([128, Bc], BF, tag="gsi")
                          gsf = Pt.tile([128, Bc], BF, tag="gsf")
                          gsg = Pt.tile([128, Bc], BF, tag="gsg")
                          gso = Pt.tile([128, Bc], BF, tag="gso")
                          nc.vector.tensor_add(gsi[:], ps[:, 0, :], t_gz[:, mi, cs])
                          nc.vector.tensor_add(gsf[:], ps[:, 1, :], t_gz[:, mf, cs])
                          nc.vector.tensor_add(gsg[:], ps[:, 2, :], t_gz[:, mg, cs])
                          nc.vector.tensor_add(gso[:], ps[:, 3, :], t_gz[:, mo, cs])
                          nc.scalar.activation(ti[:], gsi[:], AF.Sigmoid)
                          nc.scalar.activation(tf[:], gsf[:], AF.Sigmoid)
                          nc.scalar.activation(tg[:], gsg[:], AF.Tanh)
                          nc.scalar.activation(to[:], gso[:], AF.Sigmoid)
                          nc.vector.tensor_mul(tm1[:], ti[:], tg[:])
                          nc.vector.tensor_mul(tm2[:], tf[:], t_cc[:, p, :])
                          nc.vector.tensor_add(t_cc[:, p, :], tm1[:], tm2[:])
                      nc.scalar.activation(tcn[:], t_cc[:, p, :], AF.Tanh)
                      nc.vector.tensor_mul(t_emb[:, p, cs], to[:], tcn[:])


        # ge persists through the decoder (allocated after conductor frees)
        Pge = ctx.enter_context(tc.tile_pool(name="gepool", bufs=1))
        t_ge = Pge.tile([128, G, R], BF, tag="ge")

        # decoder weights (bulk, SWDGE queues; overlap with ge phase)
        Pw = ctx.enter_context(tc.tile_pool(name="wdec", bufs=1))
        t_dwn = Pw.tile([128, TK, 4 * H], F8, tag="dwn")
        nc.gpsimd.dma_start(t_dwn[:], d["dwn"][:])
        t_dwhh = Pw.tile([128, HK, 4 * H], F8, tag="dwhh")
        nc.gpsimd.dma_start(t_dwhh[:], d["dwhh"][:])
        t_owt = Pw.tile([128, HK, T], F8, tag="owt")
        nc.gpsimd.dma_start(t_owt[:], d["owt"][:])

        # ---------------- ge = emb @ dec_Wih[:, :H].T + dec_b ----------------
        with tc.tile_pool(name="gew", bufs=4) as Pgw, \
             tc.tile_pool(name="geps", bufs=2, space="PSUM") as PSg:
            t_dbias = Pgw.tile([1, 4 * H], BF, tag="dbias", bufs=1)
            nc.sync.dma_start(t_dbias[:], d["dbias"][:])
            for m in range(G):
                ms = slice(0, 128)
                wt = Pgw.tile([128, HK, 128], F8, tag="dwe")
                nc.sync.dma_start(wt[:], d["dwe"][m].rearrange(
                    "p (k j) -> p k j", k=HK))
                ps = PSg.tile([128, R], F32, tag="gep")
                nc.tensor.matmul(ps[:], t_dbias[0:1, m * 128:(m + 1) * 128],
                                 t_ones[:], start=True, stop=False)
                for k in range(0, HK, 2):
                    _mm_dr(nc, ps[:], wt, t_emb, k, ms, False, (k == HK - 2))
                nc.vector.tensor_copy(t_ge[:, m, :], ps[:])

        # ---------------- decoder: 16 note steps over 512 rows --------------
        with tc.tile_pool(name="dtmp", bufs=2) as Pdt, \
             tc.tile_pool(name="dps", bufs=3, space="PSUM") as PSd, \
             tc.tile_pool(name="dpso", bufs=2, space="PSUM") as PSo:
            for _drep in range(DEC_REPS):
              for t in range(NS):
                  hin = t_h[t % 2]
                  hout = t_h[(t + 1) % 2]
                  for p in range(HK):
                      psA = PSd.tile([128, 2, R], F32, tag="dgp", name="psA")
                      psB = PSd.tile([128, 2, R], F32, tag="dgp", name="psB")
                      for gi, m in enumerate((p, HK + p, 2 * HK + p,
                                              3 * HK + p)):
                          pst = psA if gi < 2 else psB
                          sl = gi % 2
                          ms = slice(m * 128, (m + 1) * 128)
                          for k in range(0, HK, 2):
                              _mm_dr(nc, pst[:, sl, :], t_dwhh, hin, k, ms,
                                     (k == 0), (k == HK - 2 and t == 0))
                          if t > 0:
                              for k in range(0, TK, 2):
                                  _mm_dr(nc, pst[:, sl, :], t_dwn, t_note8,
                                         k, ms, False, (k == TK - 2))
                      gsi = Pdt.tile([128, R], BF, tag="gsi")
                      gsf = Pdt.tile([128, R], BF, tag="gsf")
                      gsg = Pdt.tile([128, R], BF, tag="gsg")
                      gso = Pdt.tile([128, R], BF, tag="gso")
                      nc.vector.tensor_add(gsi[:], psA[:, 0, :], t_ge[:, p, :])
                      nc.vector.tensor_add(gsf[:], psA[:, 1, :],
                                           t_ge[:, HK + p, :])
                      nc.vector.tensor_add(gsg[:], psB[:, 0, :],
                                           t_ge[:, 2 * HK + p, :])
                      nc.vector.tensor_add(gso[:], psB[:, 1, :],
                                           t_ge[:, 3 * HK + p, :])
                      ti = Pdt.tile([128, R], BF, tag="ti")
                      tf = Pdt.tile([128, R], BF, tag="tf")
                      tg = Pdt.tile([128, R], BF, tag="tg")
                      to = Pdt.tile([128, R], BF, tag="to")
                      tcn = Pdt.tile([128, R], BF, tag="tcn")
                      tm1 = Pdt.tile([128, R], BF, tag="tm1")
                      tm2 = Pdt.tile([128, R], F32, tag="tm2")
                      nc.scalar.activation(ti[:], gsi[:], AF.Sigmoid)
                      nc.scalar.activation(tf[:], gsf[:], AF.Sigmoid)
                      nc.scalar.activation(tg[:], gsg[:], AF.Tanh)
                      nc.scalar.activation(to[:], gso[:], AF.Sigmoid)
                      nc.vector.tensor_mul(tm1[:], ti[:], tg[:])
                      nc.vector.tensor_mul(tm2[:], tf[:], t_c[:, p, :])
                      nc.vector.tensor_add(t_c[:, p, :], tm1[:], tm2[:])
                      nc.scalar.activation(tcn[:], t_c[:, p, :], AF.Tanh)
                      nc.vector.tensor_mul(hout[:, p, :], to[:], tcn[:])
                  # output projection + sigmoid -> note (bf16 out + fp8 mirror)
                  for tk in range(TK):
                      ts_ = slice(tk * 128, (tk + 1) * 128)
                      po = PSo.tile([128, R], F32, tag="dpo")
                      for k in range(0, HK, 2):
                          _mm_dr(nc, po[:], t_owt, hout, k, ts_,
                                 (k == 0), (k == HK - 2))
                      nc.scalar.activation(t_note[:, tk, :], po[:],
                                           AF.Sigmoid, bias=t_ob[:, tk:tk + 1])
                      nc.scalar.activation(t_note8[:, tk, :], po[:],
                                           AF.Sigmoid, bias=t_ob[:, tk:tk + 1])
                      nc.sync.dma_start(d["outbuf"][t, tk], t_note[:, tk, :])


import os
DEC_REPS = int(os.environ.get("KBENCH_DEC_REPS", "1"))
COND_REPS = int(os.environ.get("KBENCH_COND_REPS", "1"))

_CACHE = {}


def _build():
    if "nc" not in _CACHE:
        nc = bacc.Bacc("TRN2", target_bir_lowering=False, debug=False,
                       num_devices=NCORES)
        d = _declare(nc)
        with tile.TileContext(nc) as tc:
            _body(nc, tc, d)
        nc.compile()
        _CACHE["nc"] = nc
    return _CACHE["nc"]


def _feat_major(W, dt):
    """[J, K] -> [128, K/128, J] (stationary lhsT chunk layout)."""
    J, K = W.shape
    return np.ascontiguousarray(
        W.reshape(J, K // 128, 128).transpose(2, 1, 0)).astype(dt)


def _pack_inputs(inputs):
    z = np.asarray(inputs["z"], np.float32)
    dec_h0 = np.asarray(inputs["dec_h0"], np.float32)
    dec_c0 = np.asarray(inputs["dec_c0"], np.float32)
    cond_b = np.asarray(inputs["cond_bih"] + inputs["cond_bhh"], np.float32)
    dec_b = np.asarray(inputs["dec_bih"] + inputs["dec_bhh"], np.float32)
    out_b = np.asarray(inputs["out_b"], np.float32)

    shared = {
        "ones": np.ones((1, R), dtype=bf16),
        "cbias": cond_b[None, :].astype(bf16),
        "dbias": dec_b[None, :].astype(bf16),
        "obias": np.ascontiguousarray(out_b.reshape(TK, 128).T).astype(np.float32),
        "cwih": _feat_major(np.asarray(inputs["cond_Wih"], np.float32), f8),
        "cwhh": _feat_major(np.asarray(inputs["cond_Whh"], np.float32), f8),
        "dwn": _feat_major(np.asarray(inputs["dec_Wih"][:, H:], np.float32), f8),
        "dwhh": _feat_major(np.asarray(inputs["dec_Whh"], np.float32), f8),
        "owt": _feat_major(np.asarray(inputs["out_W"], np.float32), f8),
    }
    dwe_fm = _feat_major(np.asarray(inputs["dec_Wih"][:, :H], np.float32), f8)
    # slab m: [128, HK*128] so each DMA is one contiguous 128KB read
    shared["dwe"] = np.ascontiguousarray(
        dwe_fm.reshape(128, HK, G, 128).transpose(2, 0, 1, 3).reshape(
            G, 128, HK * 128))

    z_lv = z[:, np.arange(L) * L, 0, :]           # [B, L, Z]
    in_maps = []
    for c in range(NCORES):
        bs = slice(c * Bc, (c + 1) * Bc)
        zc = z_lv[bs]                              # [Bc, L, Z]
        zT = np.ascontiguousarray(
            zc.reshape(Bc, L, ZK, 128).transpose(3, 2, 1, 0).reshape(128, ZK, R)
        ).astype(f8)
        h0 = dec_h0[:, bs, :]                      # [L, Bc, H]
        h0T = np.ascontiguousarray(
            h0.reshape(L, Bc, HK, 128).transpose(3, 2, 0, 1).reshape(128, HK, R))
        c0 = dec_c0[:, bs, :]
        c0T = np.ascontiguousarray(
            c0.reshape(L, Bc, HK, 128).transpose(3, 2, 0, 1).reshape(128, HK, R))
        m = dict(shared)
        m["zT"] = zT
        m["h0T"] = h0T.astype(f8)
        m["c0T"] = c0T.astype(np.float32)
        in_maps.append(m)
    return in_maps


def _unpack_outputs(core_outs):
    notes = np.empty((B, L * NS, T), np.float32)
    for c, arr in enumerate(core_outs):
        # arr [NS, TK, 128, R] -> [Bc, L, NS, T]
        a = arr.astype(np.float32).reshape(NS, TK, 128, L, Bc).transpose(4, 3, 0, 1, 2)
        notes[c * Bc:(c + 1) * Bc] = a.reshape(Bc, L, NS, T).reshape(
            Bc, L * NS, T)
    return notes


def kernel(**inputs):
    nc = _build()
    in_maps = _pack_inputs(inputs)
    res = run_bass_kernel_spmd(nc, in_maps, list(range(NCORES)))
    return _unpack_outputs([r["outbuf"] for r in res.results])


# revision 34
# speedup vs baseline: 1.3505x; 1.3505x over previous
"""Trainium2 Bass kernel for nn_Decoder (MusicVAE-style hierarchical decoder).

Strategy (8 NeuronCores, data-parallel over batch, no inter-core comms):
  - Conductor LSTM (16 sequential levels, batch 32/core) computes per-level
    embeddings; decoder levels are independent, so all 16 levels are batched:
    512 decoder rows per core, 16 sequential note steps.
  - fp8(e4m3) matmuls, DoubleRow perf mode for 512-row streams; fp32 PSUM.
  - Gate chunks are laid out p-adjacent in order (i, f, o, g) so the three
    sigmoid gates batch into one activation op and (i,f) / (o,g) pairs map
    onto two 2-bank PSUM accumulation tiles.
  - The g-gate's conductor-embedding contribution (emb @ dec_Wih_g.T) is
    recomputed on the PE every step (cheaper than a DVE add at model rates);
    its bias rides the tanh activation's bias port.  i/f/o biases are folded
    into ge / gz via Identity-activation copies (no bias matmuls, no ones).
  - Elementwise work is spread across DVE (vector), Pool (gpsimd) and
    Activation engines to balance the per-step makespan against the PE.
"""
import numpy as np
import ml_dtypes

import concourse.bacc as bacc
import concourse.tile as tile
import concourse.mybir as mybir
from concourse.bass_utils import run_bass_kernel_spmd

bf16 = ml_dtypes.bfloat16
f8 = ml_dtypes.float8_e4m3
F32 = mybir.dt.float32
BF = mybir.dt.bfloat16
F8 = mybir.dt.float8e4
AF = mybir.ActivationFunctionType
DR = mybir.MatmulPerfMode.DoubleRow

NCORES = 8
B, Z, H, T = 256, 512, 1024, 512
L, NS = 16, 16
Bc = B // NCORES            # 32 batch rows per core
R = L * Bc                  # 512 decoder rows per core (levels x batch)
HK, TK, ZK = H // 128, T // 128, Z // 128   # 8, 4, 4
G = 4 * H // 128            # 32 gate chunks of 128


def _declare(nc):
    d = {}
    ei = dict(kind="ExternalInput")
    d["cbt"] = nc.dram_tensor("cbt", [128, G], F32, **ei)
    d["dbg"] = nc.dram_tensor("dbg", [128, HK], F32, **ei)
    d["obias"] = nc.dram_tensor("obias", [128, TK], F32, **ei)
    d["zT"] = nc.dram_tensor("zT", [128, ZK, R], F8, **ei)
    d["h0T"] = nc.dram_tensor("h0T", [128, HK, R], F8, **ei)
    d["c0T"] = nc.dram_tensor("c0T", [128, HK, R], F32, **ei)
    d["cwih"] = nc.dram_tensor("cwih", [128, ZK, 4 * H], F8, **ei)
    d["cwhh"] = nc.dram_tensor("cwhh", [128, HK, 4 * H], F8, **ei)
    d["ones"] = nc.dram_tensor("ones", [1, Bc], BF, **ei)
    d["gebb"] = nc.dram_tensor("gebb", [1, 3 * H], BF, **ei)
    d["dwe"] = nc.dram_tensor("dwe", [128, HK, 3 * H], F8, **ei)
    d["dweg"] = nc.dram_tensor("dweg", [128, HK, H], F8, **ei)
    d["dwn"] = nc.dram_tensor("dwn", [128, TK, 4 * H], F8, **ei)
    d["dwhh"] = nc.dram_tensor("dwhh", [128, HK, 4 * H], F8, **ei)
    d["owt"] = nc.dram_tensor("owt", [128, HK, T], F8, **ei)
    d["outbuf"] = nc.dram_tensor("outbuf", [NS, TK, 128, R], BF,
                                 kind="ExternalOutput")
    return d


def _mm_dr(nc, out, w3, x3, ks, ms, start, stop):
    """DoubleRow fp8 matmul over k-subtile pair (ks, ks+1)."""
    return nc.tensor.matmul(out, w3[:, ks:ks + 2, ms], x3[:, ks:ks + 2, :],
                            start=start, stop=stop, perf_mode=DR)


PHASE_MARKS = []


def _mark(nc, name):
    try:
        PHASE_MARKS.append((name, sum(1 for _ in nc.all_instructions())))
    except Exception:
        pass


def _body(nc, tc, d):
    import contextlib
    with contextlib.ExitStack() as ctx:
        Pp = ctx.enter_context(tc.tile_pool(name="persist", bufs=1))

        t_ob = Pp.tile([128, TK], F32, tag="obias")
        t_dbg = Pp.tile([128, HK], F32, tag="dbg")
        t_emb = Pp.tile([128, HK, R], F8, tag="emb")
        t_h = [Pp.tile([128, HK, R], F8, tag=f"hT{i}", name=f"hT{i}")
               for i in (0, 1)]
        t_c = Pp.tile([128, HK, R], F32, tag="c")
        t_note = Pp.tile([128, TK, R], BF, tag="note")
        t_note8 = Pp.tile([128, TK, R], F8, tag="note8")
        # ge persists through the decoder; filled per-level in the conductor
        t_ge = Pp.tile([128, 3 * HK, R], BF, tag="ge")
        # decoder weights needed at dec00 (loaded during the conductor)
        t_dwhh = Pp.tile([128, HK, 4 * H], F8, tag="dwhh")
        t_dweg = Pp.tile([128, HK, H], F8, tag="dweg")

        # ---------------- conductor (+ per-level ge fill) ----------------
        with tc.tile_pool(name="cond", bufs=1) as Pc, \
             tc.tile_pool(name="ctmp", bufs=2) as Pt:
            # DMA order on the shared engine: gz deps first, then cwhh
            # (needed at level 1), then dwe (ge fills), then the rest.
            t_cwhh = Pc.tile([128, HK, 4 * H], F8, tag="cwhh")
            t_ones = Pc.tile([1, Bc], BF, tag="ones")
            t_gebb = Pc.tile([1, 3 * H], BF, tag="gebb")
            # gz chunks indexed [p, gate] with gate order (i, f, o, g)
            t_gz = Pc.tile([128, HK, 4, R], BF, tag="gz")
            t_cc = Pc.tile([128, HK, Bc], F32, tag="cc")

            # gz = z @ cond_Wih.T + cond_b for all levels at once (fp8 DR);
            # bias folded via Identity-activation copy from PSUM.
            _mark(nc, "gz")
            with tc.tile_pool(name="condz", bufs=1) as Pcz, \
                 tc.tile_pool(name="gzps", bufs=2, space="PSUM") as PSz:
                t_cwih = Pcz.tile([128, ZK, 4 * H], F8, tag="cwih")
                nc.sync.dma_start(t_cwih[:], d["cwih"][:])
                t_zT = Pcz.tile([128, ZK, R], F8, tag="zT")
                nc.sync.dma_start(t_zT[:], d["zT"][:])
                t_cbt = Pcz.tile([128, G], F32, tag="cbt")
                nc.sync.dma_start(t_cbt[:], d["cbt"][:])
                nc.sync.dma_start(t_cwhh[:], d["cwhh"][:])
                nc.sync.dma_start(t_ones[:], d["ones"][:])
                nc.sync.dma_start(t_gebb[:], d["gebb"][:])
                for m in range(G):
                    ms = slice(m * 128, (m + 1) * 128)
                    ps = PSz.tile([128, R], F32, tag="gzp", name="gzp")
                    for k in range(0, ZK, 2):
                        _mm_dr(nc, ps[:], t_cwih, t_zT, k, ms,
                               (k == 0), (k == ZK - 2))
                    nc.scalar.activation(t_gz[:, m // 4, m % 4, :], ps[:],
                                         AF.Identity, bias=t_cbt[:, m:m + 1])

            # sequential levels (fp8 non-DR: 32-row streams), elementwise
            # batched level-wide via strided views; each level's ge slice is
            # computed on the PE right after its emb is ready, filling the
            # PE idle while the next level's elementwise chain runs.
            _mark(nc, "conductor")
            with tc.tile_pool(name="dwepool", bufs=1) as Pdwe, \
                 tc.tile_pool(name="cps", bufs=2, space="PSUM") as PSc, \
                 tc.tile_pool(name="geps", bufs=2, space="PSUM") as PSg:
                # dwe lives only through the conductor levels (ge fills)
                t_dwe = Pdwe.tile([128, HK, 3 * H], F8, tag="dwe")
                nc.sync.dma_start(t_dwe[:], d["dwe"][:])
                # decoder weights/state needed at dec00 load during the levels
                nc.sync.dma_start(t_h[0][:], d["h0T"][:])
                nc.sync.dma_start(t_c[:], d["c0T"][:])
                nc.sync.dma_start(t_ob[:], d["obias"][:])
                nc.sync.dma_start(t_dbg[:], d["dbg"][:])
                nc.sync.dma_start(t_dwhh[:], d["dwhh"][:])
                nc.sync.dma_start(t_dweg[:], d["dweg"][:])

                def ge_fill(lv):
                    cs = slice(lv * Bc, (lv + 1) * Bc)
                    gp = PSg.tile([128, 3 * HK, Bc], F32, tag="gep",
                                  name="gep")
                    for m in range(3 * HK):
                        nc.tensor.matmul(gp[:, m, :],
                                         t_gebb[0:1, m * 128:(m + 1) * 128],
                                         t_ones[:], start=True, stop=False)
                        for k in range(HK):
                            nc.tensor.matmul(
                                gp[:, m, :], t_dwe[:, k, m * 128:(m + 1) * 128],
                                t_emb[:, k, cs],
                                start=False, stop=(k == HK - 1))
                    nc.vector.tensor_copy(t_ge[:, :, cs], gp[:])

                for _crep in range(COND_REPS):
                  for lv in range(L):
                      cs = slice(lv * Bc, (lv + 1) * Bc)
                      ps_prev = slice((lv - 1) * Bc, lv * Bc)
                      tsig = Pt.tile([128, HK, 3, Bc], BF, tag="tsig",
                                     name="tsig")
                      tg = Pt.tile([128, HK, Bc], BF, tag="tg", name="tg")
                      tcn = Pt.tile([128, HK, Bc], BF, tag="tcn", name="tcn")
                      if lv == 0:
                          # h0 == 0: gates are just gz; c0 == 0
                          nc.scalar.activation(tsig[:], t_gz[:, :, 0:3, cs],
                                               AF.Sigmoid)
                          nc.scalar.activation(tg[:], t_gz[:, :, 3, cs],
                                               AF.Tanh)
                          nc.vector.tensor_mul(t_cc[:], tsig[:, :, 0, :],
                                               tg[:])
                      else:
                          ps = PSc.tile([128, HK, 4, Bc], F32, tag="cgp",
                                        name="cgp")
                          for p in range(HK):
                              for g in range(4):
                                  ms = slice((p * 4 + g) * 128,
                                             (p * 4 + g + 1) * 128)
                                  for k in range(HK):
                                      nc.tensor.matmul(
                                          ps[:, p, g, :], t_cwhh[:, k, ms],
                                          t_emb[:, k, ps_prev],
                                          start=(k == 0), stop=(k == HK - 1))
                          # previous level's ge fills the PE while this
                          # level's elementwise chain runs
                          ge_fill(lv - 1)
                          gs = Pt.tile([128, HK, 4, Bc], BF, tag="gs",
                                       name="gs")
                          nc.vector.tensor_add(gs[:], ps[:],
                                               t_gz[:, :, :, cs])
                          nc.scalar.activation(tsig[:], gs[:, :, 0:3, :],
                                               AF.Sigmoid)
                          nc.scalar.activation(tg[:], gs[:, :, 3, :], AF.Tanh)
                          tm1 = Pt.tile([128, HK, Bc], BF, tag="tm1",
                                        name="tm1")
                          tm2 = Pt.tile([128, HK, Bc], F32, tag="tm2",
                                        name="tm2")
                          nc.vector.tensor_mul(tm1[:], tsig[:, :, 0, :],
                                               tg[:])
                          nc.gpsimd.tensor_mul(tm2[:], tsig[:, :, 1, :],
                                               t_cc[:])
                          nc.vector.tensor_add(t_cc[:], tm1[:], tm2[:])
                      nc.scalar.activation(tcn[:], t_cc[:], AF.Tanh)
                      nc.vector.tensor_mul(t_emb[:, :, cs], tsig[:, :, 2, :],
                                           tcn[:])
                  ge_fill(L - 1)

        # remaining decoder weights (first used at dec00 outproj / dec01)
        Pw2 = ctx.enter_context(tc.tile_pool(name="wdec2", bufs=1))
        t_dwn = Pw2.tile([128, TK, 4 * H], F8, tag="dwn")
        nc.sync.dma_start(t_dwn[:], d["dwn"][:])
        t_owt = Pw2.tile([128, HK, T], F8, tag="owt")
        nc.sync.dma_start(t_owt[:], d["owt"][:])

        # ---------------- decoder: 16 note steps over 512 rows --------------
        with tc.tile_pool(name="dtmp", bufs=4) as Pdt, \
             tc.tile_pool(name="dps", bufs=3, space="PSUM") as PSd, \
             tc.tile_pool(name="dpso", bufs=2, space="PSUM") as PSo:
            prefetched = {}
            for _drep in range(DEC_REPS):
              for t in range(NS):
                  _mark(nc, f"dec{t:02d}")
                  hin = t_h[t % 2]
                  hout = t_h[(t + 1) % 2]
                  psAB = {}
                  # software-pipelined emission: stage ops of chunk p are
                  # emitted after stage ops of chunk p+1's predecessors so
                  # each engine's FIFO never head-of-line blocks on a
                  # dependency that a later-emitted independent op could fill.
                  tiles = {}

                  def mms(p):
                      psA = PSd.tile([128, 2, R], F32, tag="dgp", name="psA")
                      if p in prefetched:
                          psB = prefetched.pop(p)
                      else:
                          psB = PSd.tile([128, 2, R], F32, tag="dgp",
                                         name="psB")
                          # g gate: emb contribution recomputed on PE
                          for k in range(0, HK, 2):
                              _mm_dr(nc, psB[:, 1, :], t_dweg, t_emb,
                                     k, slice(p * 128, (p + 1) * 128),
                                     (k == 0), False)
                      psAB[p] = (psA, psB)
                      for gi in range(4):
                          pst = psA if gi < 2 else psB
                          sl = gi % 2
                          ms = slice((p * 4 + gi) * 128,
                                     (p * 4 + gi + 1) * 128)
                          for k in range(0, HK, 2):
                              _mm_dr(nc, pst[:, sl, :], t_dwhh, hin, k, ms,
                                     (k == 0 and gi != 3),
                                     (t == 0 and k == HK - 2))
                          if t > 0:
                              for k in range(0, TK, 2):
                                  _mm_dr(nc, pst[:, sl, :], t_dwn, t_note8,
                                         k, ms, False, (k == TK - 2))

                  def adds(p):
                      psA, psB = psAB[p]
                      gs3 = Pdt.tile([128, 3, R], BF, tag="gs3", name="gs3")
                      tg = Pdt.tile([128, R], BF, tag="tg", name="tg")
                      tiles[p] = (gs3, tg)
                      nc.vector.tensor_add(gs3[:, 0:2, :], psA[:],
                                           t_ge[:, 3 * p:3 * p + 2, :])
                      nc.vector.tensor_add(gs3[:, 2, :], psB[:, 0, :],
                                           t_ge[:, 3 * p + 2, :])
                      nc.scalar.activation(tg[:], psB[:, 1, :], AF.Tanh,
                                           bias=t_dbg[:, p:p + 1])

                  def acts(p):
                      gs3, tg = tiles[p]
                      tsig = Pdt.tile([128, 3, R], BF, tag="tsig3",
                                      name="tsig3")
                      tiles[p] = (tsig, tg)
                      nc.scalar.activation(tsig[:], gs3[:], AF.Sigmoid)

                  def tail(p):
                      tsig, tg = tiles.pop(p)
                      tcn = Pdt.tile([128, R], BF, tag="tcn", name="tcn")
                      tm1 = Pdt.tile([128, R], BF, tag="tm1", name="tm1")
                      tm2 = Pdt.tile([128, R], F32, tag="tm2", name="tm2")
                      nc.vector.tensor_mul(tm1[:], tsig[:, 0, :], tg[:])
                      nc.gpsimd.tensor_mul(tm2[:], tsig[:, 1, :],
                                           t_c[:, p, :])
                      nc.gpsimd.tensor_add(t_c[:, p, :], tm1[:], tm2[:])
                      nc.scalar.activation(tcn[:], t_c[:, p, :], AF.Tanh)
                      nc.vector.tensor_mul(hout[:, p, :], tsig[:, 2, :],
                                           tcn[:])

                  for p in range(HK + 3):
                      if p < HK:
                          mms(p)
                          adds(p)
                      if 1 <= p - 0 and p - 1 < HK:
                          acts(p - 1)
                      if p >= 3:
                          tail(p - 3)
                  # prefetch next step's g-gate emb matmuls ahead of the
                  # outproj in the PE FIFO: they are ready immediately and
                  # fill the tail wait, keeping the PE p-state warm
                  if t + 1 < NS:
                      for p in range(2):
                          psB = PSd.tile([128, 2, R], F32, tag="dgp",
                                         name="psBpre")
                          for k in range(0, HK, 2):
                              _mm_dr(nc, psB[:, 1, :], t_dweg, t_emb, k,
                                     slice(p * 128, (p + 1) * 128),
                                     (k == 0), False)
                          prefetched[p] = psB
                  # output projection + sigmoid -> note (fp8 mirror first so
                  # the next step's Wn matmuls unblock as early as possible)
                  for tk in range(TK):
                      ts_ = slice(tk * 128, (tk + 1) * 128)
                      po = PSo.tile([128, R], F32, tag="dpo", name="dpo")
                      for k in range(0, HK, 2):
                          _mm_dr(nc, po[:], t_owt, hout, k, ts_,
                                 (k == 0), (k == HK - 2))
                      nc.scalar.activation(t_note8[:, tk, :], po[:],
                                           AF.Sigmoid, bias=t_ob[:, tk:tk + 1])
                      nc.scalar.activation(t_note[:, tk, :], po[:],
                                           AF.Sigmoid, bias=t_ob[:, tk:tk + 1])
                      nc.sync.dma_start(d["outbuf"][t, tk], t_note[:, tk, :])


import os
DEC_REPS = int(os.environ.get("KBENCH_DEC_REPS", "1"))
COND_REPS = int(os.environ.get("KBENCH_COND_REPS", "1"))

_CACHE = {}


def _build():
    if "nc" not in _CACHE:
        nc = bacc.Bacc("TRN2", target_bir_lowering=False, debug=False,
                       num_devices=NCORES)
        d = _declare(nc)
        with tile.TileContext(nc) as tc:
            _body(nc, tc, d)
        nc.compile()
        _CACHE["nc"] = nc
    return _CACHE["nc"]


def _feat_major(W, dt):
    """[J, K] -> [128, K/128, J] (stationary lhsT chunk layout)."""
    J, K = W.shape
    return np.ascontiguousarray(
        W.reshape(J, K // 128, 128).transpose(2, 1, 0)).astype(dt)


def _reorder4(W):
    """[4H, K] rows in PyTorch gate blocks (i,f,g,o) -> p-adjacent chunks in
    order (i,f,o,g): new chunk m = p*4 + {0:i,1:f,2:o,3:g}."""
    K = W.shape[1]
    W4 = W.reshape(4, HK, 128, K)[[0, 1, 3, 2]]
    return np.ascontiguousarray(W4.transpose(1, 0, 2, 3).reshape(4 * H, K))


def _pack_inputs(inputs):
    z = np.asarray(inputs["z"], np.float32)
    dec_h0 = np.asarray(inputs["dec_h0"], np.float32)
    dec_c0 = np.asarray(inputs["dec_c0"], np.float32)
    cond_b = np.asarray(inputs["cond_bih"] + inputs["cond_bhh"], np.float32)
    dec_b = np.asarray(inputs["dec_bih"] + inputs["dec_bhh"], np.float32)
    out_b = np.asarray(inputs["out_b"], np.float32)

    cb4 = cond_b.reshape(4, HK, 128)[[0, 1, 3, 2]]
    cbt = np.ascontiguousarray(cb4.transpose(1, 0, 2).reshape(G, 128).T)
    db4 = dec_b.reshape(4, HK, 128)
    gebb = np.ascontiguousarray(
        db4[[0, 1, 3]].transpose(1, 0, 2).reshape(1, 3 * H))
    dbg = np.ascontiguousarray(db4[2].T)

    We = np.asarray(inputs["dec_Wih"][:, :H], np.float32)
    We4 = We.reshape(4, HK, 128, H)
    We3 = np.ascontiguousarray(
        We4[[0, 1, 3]].transpose(1, 0, 2, 3).reshape(3 * H, H))
    dwe = _feat_major(We3, f8)                      # [128, HK, 3H]
    dweg = _feat_major(np.ascontiguousarray(We4[2].reshape(H, H)), f8)

    shared = {
        "cbt": cbt.astype(np.float32),
        "dbg": dbg.astype(np.float32),
        "ones": np.ones((1, Bc), dtype=bf16),
        "gebb": gebb.astype(bf16),
        "obias": np.ascontiguousarray(out_b.reshape(TK, 128).T).astype(np.float32),
        "cwih": _feat_major(_reorder4(np.asarray(inputs["cond_Wih"], np.float32)), f8),
        "cwhh": _feat_major(_reorder4(np.asarray(inputs["cond_Whh"], np.float32)), f8),
        "dwn": _feat_major(_reorder4(np.asarray(inputs["dec_Wih"][:, H:], np.float32)), f8),
        "dwhh": _feat_major(_reorder4(np.asarray(inputs["dec_Whh"], np.float32)), f8),
        "owt": _feat_major(np.asarray(inputs["out_W"], np.float32), f8),
        "dwe": dwe,
        "dweg": dweg,
    }

    z_lv = z[:, np.arange(L) * L, 0, :]           # [B, L, Z]
    in_maps = []
    for c in range(NCORES):
        bs = slice(c * Bc, (c + 1) * Bc)
        zc = z_lv[bs]                              # [Bc, L, Z]
        zT = np.ascontiguousarray(
            zc.reshape(Bc, L, ZK, 128).transpose(3, 2, 1, 0).reshape(128, ZK, R)
        ).astype(f8)
        h0 = dec_h0[:, bs, :]                      # [L, Bc, H]
        h0T = np.ascontiguousarray(
            h0.reshape(L, Bc, HK, 128).transpose(3, 2, 0, 1).reshape(128, HK, R))
        c0 = dec_c0[:, bs, :]
        c0T = np.ascontiguousarray(
            c0.reshape(L, Bc, HK, 128).transpose(3, 2, 0, 1).reshape(128, HK, R))
        m = dict(shared)
        m["zT"] = zT
        m["h0T"] = h0T.astype(f8)
        m["c0T"] = c0T.astype(np.float32)
        in_maps.append(m)
    return in_maps


def _unpack_outputs(core_outs):
    notes = np.empty((B, L * NS, T), np.float32)
    for c, arr in enumerate(core_outs):
        # arr [NS, TK, 128, R] -> [Bc, L, NS, T]
        a = arr.astype(np.float32).reshape(NS, TK, 128, L, Bc).transpose(4, 3, 0, 1, 2)
        notes[c * Bc:(c + 1) * Bc] = a.reshape(Bc, L, NS, T).reshape(
            Bc, L * NS, T)
    return notes


def kernel(**inputs):
    nc = _build()
    in_maps = _pack_inputs(inputs)
    res = run_bass_kernel_spmd(nc, in_maps, list(range(NCORES)))
    return _unpack_outputs([r["outbuf"] for r in res.results])


# revision 47
# speedup vs baseline: 1.4881x; 1.1018x over previous
"""Trainium2 Bass kernel for nn_Decoder (MusicVAE-style hierarchical decoder).

Strategy (8 NeuronCores, data-parallel over batch, no inter-core comms):
  - Conductor LSTM (16 sequential levels, batch 32/core) computes per-level
    embeddings; decoder levels are independent, so all 16 levels are batched:
    512 decoder rows per core, 16 sequential note steps.
  - fp8(e4m3) matmuls, DoubleRow perf mode for 512-row streams; fp32 PSUM.
  - Gate chunks are laid out p-adjacent in order (i, f, o, g) so the three
    sigmoid gates batch into one activation op and (i,f) / (o,g) pairs map
    onto two 2-bank PSUM accumulation tiles.
  - The g-gate's conductor-embedding contribution (emb @ dec_Wih_g.T) is
    recomputed on the PE every step (cheaper than a DVE add at model rates);
    its bias rides the tanh activation's bias port.  i/f/o biases are folded
    into ge / gz via Identity-activation copies (no bias matmuls, no ones).
  - Elementwise work is spread across DVE (vector), Pool (gpsimd) and
    Activation engines to balance the per-step makespan against the PE.
"""
import numpy as np
import ml_dtypes

import concourse.bacc as bacc
import concourse.tile as tile
import concourse.mybir as mybir
from concourse.bass_utils import run_bass_kernel_spmd

bf16 = ml_dtypes.bfloat16
f8 = ml_dtypes.float8_e4m3
F32 = mybir.dt.float32
BF = mybir.dt.bfloat16
F8 = mybir.dt.float8e4
AF = mybir.ActivationFunctionType
DR = mybir.MatmulPerfMode.DoubleRow

NCORES = 8
B, Z, H, T = 256, 512, 1024, 512
L, NS = 16, 16
Bc = B // NCORES            # 32 batch rows per core
R = L * Bc                  # 512 decoder rows per core (levels x batch)
HK, TK, ZK = H // 128, T // 128, Z // 128   # 8, 4, 4
G = 4 * H // 128            # 32 gate chunks of 128


def _declare(nc):
    d = {}
    ei = dict(kind="ExternalInput")
    d["cbt"] = nc.dram_tensor("cbt", [128, G], F32, **ei)
    d["dbg"] = nc.dram_tensor("dbg", [128, HK], F32, **ei)
    d["obias"] = nc.dram_tensor("obias", [128, TK], F32, **ei)
    d["zT"] = nc.dram_tensor("zT", [128, ZK, R], F8, **ei)
    d["h0T"] = nc.dram_tensor("h0T", [128, HK, R], F8, **ei)
    d["c0T"] = nc.dram_tensor("c0T", [128, HK, R], F32, **ei)
    d["cwih"] = nc.dram_tensor("cwih", [128, ZK, 4 * H], F8, **ei)
    d["cwhh"] = nc.dram_tensor("cwhh", [128, HK, 4 * H], F8, **ei)
    d["ones"] = nc.dram_tensor("ones", [1, Bc], BF, **ei)
    d["gebb"] = nc.dram_tensor("gebb", [1, 3 * H], BF, **ei)
    d["dwe"] = nc.dram_tensor("dwe", [128, HK, 3 * H], F8, **ei)
    d["dweg"] = nc.dram_tensor("dweg", [128, HK, H], F8, **ei)
    d["dwn"] = nc.dram_tensor("dwn", [128, TK, 4 * H], F8, **ei)
    d["dwhh"] = nc.dram_tensor("dwhh", [128, HK, 4 * H], F8, **ei)
    d["owt"] = nc.dram_tensor("owt", [128, HK, T], F8, **ei)
    d["outbuf"] = nc.dram_tensor("outbuf", [NS, TK, 128, R], BF,
                                 kind="ExternalOutput")
    return d


def _mm_dr(nc, out, w3, x3, ks, ms, start, stop):
    """DoubleRow fp8 matmul over k-subtile pair (ks, ks+1)."""
    return nc.tensor.matmul(out, w3[:, ks:ks + 2, ms], x3[:, ks:ks + 2, :],
                            start=start, stop=stop, perf_mode=DR)


PHASE_MARKS = []


def _mark(nc, name):
    try:
        PHASE_MARKS.append((name, sum(1 for _ in nc.all_instructions())))
    except Exception:
        pass


def _body(nc, tc, d):
    import contextlib
    with contextlib.ExitStack() as ctx:
        Pp = ctx.enter_context(tc.tile_pool(name="persist", bufs=1))

        t_ob = Pp.tile([128, TK], F32, tag="obias")
        t_dbg = Pp.tile([128, HK], F32, tag="dbg")
        t_emb = Pp.tile([128, HK, R], F8, tag="emb")
        t_h = [Pp.tile([128, HK, R], F8, tag=f"hT{i}", name=f"hT{i}")
               for i in (0, 1)]
        t_c = Pp.tile([128, HK, R], F32, tag="c")
        t_note = Pp.tile([128, TK, R], BF, tag="note")
        t_note8 = Pp.tile([128, TK, R], F8, tag="note8")
        # ge persists through the decoder; filled per-level in the conductor
        t_ge = Pp.tile([128, 3 * HK, R], BF, tag="ge")
        # decoder weights needed at dec00 (loaded during the conductor)
        t_dwhh = Pp.tile([128, HK, 4 * H], F8, tag="dwhh")
        t_dweg = Pp.tile([128, HK, H], F8, tag="dweg")

        # ---------------- conductor (+ per-level ge fill) ----------------
        with tc.tile_pool(name="cond", bufs=1) as Pc, \
             tc.tile_pool(name="ctmp", bufs=2) as Pt:
            # DMA order on the shared engine: gz deps first, then cwhh
            # (needed at level 1), then dwe (ge fills), then the rest.
            t_cwhh = Pc.tile([128, HK, 4 * H], F8, tag="cwhh")
            t_ones = Pc.tile([1, Bc], BF, tag="ones")
            t_gebb = Pc.tile([1, 3 * H], BF, tag="gebb")
            # gz chunks indexed [p, gate] with gate order (i, f, o, g)
            t_gz = Pc.tile([128, HK, 4, R], BF, tag="gz")
            t_cc = Pc.tile([128, HK, Bc], F32, tag="cc")

            # gz = z @ cond_Wih.T + cond_b for all levels at once (fp8 DR);
            # bias folded via Identity-activation copy from PSUM.
            _mark(nc, "gz")
            with tc.tile_pool(name="condz", bufs=1) as Pcz, \
                 tc.tile_pool(name="gzps", bufs=2, space="PSUM") as PSz:
                t_cwih = Pcz.tile([128, ZK, 4 * H], F8, tag="cwih")
                nc.sync.dma_start(t_cwih[:], d["cwih"][:])
                t_zT = Pcz.tile([128, ZK, R], F8, tag="zT")
                nc.sync.dma_start(t_zT[:], d["zT"][:])
                t_cbt = Pcz.tile([128, G], F32, tag="cbt")
                nc.sync.dma_start(t_cbt[:], d["cbt"][:])
                nc.sync.dma_start(t_cwhh[:], d["cwhh"][:])
                nc.sync.dma_start(t_ones[:], d["ones"][:])
                nc.sync.dma_start(t_gebb[:], d["gebb"][:])
                for m in range(G):
                    ms = slice(m * 128, (m + 1) * 128)
                    ps = PSz.tile([128, R], F32, tag="gzp", name="gzp")
                    for k in range(0, ZK, 2):
                        _mm_dr(nc, ps[:], t_cwih, t_zT, k, ms,
                               (k == 0), (k == ZK - 2))
                    nc.scalar.activation(t_gz[:, m // 4, m % 4, :], ps[:],
                                         AF.Identity, bias=t_cbt[:, m:m + 1])

            # sequential levels (fp8 non-DR: 32-row streams), elementwise
            # batched level-wide via strided views; each level's ge slice is
            # computed on the PE right after its emb is ready, filling the
            # PE idle while the next level's elementwise chain runs.
            _mark(nc, "conductor")
            with tc.tile_pool(name="dwepool", bufs=1) as Pdwe, \
                 tc.tile_pool(name="cps", bufs=2, space="PSUM") as PSc, \
                 tc.tile_pool(name="geps", bufs=2, space="PSUM") as PSg:
                # dwe lives only through the conductor levels (ge fills)
                t_dwe = Pdwe.tile([128, HK, 3 * H], F8, tag="dwe")
                nc.sync.dma_start(t_dwe[:], d["dwe"][:])
                # decoder weights/state needed at dec00 load during the levels
                nc.sync.dma_start(t_h[0][:], d["h0T"][:])
                nc.sync.dma_start(t_c[:], d["c0T"][:])
                nc.sync.dma_start(t_ob[:], d["obias"][:])
                nc.sync.dma_start(t_dbg[:], d["dbg"][:])
                nc.sync.dma_start(t_dwhh[:], d["dwhh"][:])
                nc.sync.dma_start(t_dweg[:], d["dweg"][:])

                def ge_fill(lv):
                    cs = slice(lv * Bc, (lv + 1) * Bc)
                    gp = PSg.tile([128, 3 * HK, Bc], F32, tag="gep",
                                  name="gep")
                    for m in range(3 * HK):
                        nc.tensor.matmul(gp[:, m, :],
                                         t_gebb[0:1, m * 128:(m + 1) * 128],
                                         t_ones[:], start=True, stop=False)
                        for k in range(0, HK, 2):
                            nc.tensor.matmul(
                                gp[:, m, :],
                                t_dwe[:, k:k + 2, m * 128:(m + 1) * 128],
                                t_emb[:, k:k + 2, cs],
                                start=False, stop=(k == HK - 2), perf_mode=DR)
                    nc.vector.tensor_copy(t_ge[:, :, cs], gp[:])

                for _crep in range(COND_REPS):
                  for lv in range(L):
                      cs = slice(lv * Bc, (lv + 1) * Bc)
                      ps_prev = slice((lv - 1) * Bc, lv * Bc)
                      tsig = Pt.tile([128, HK, 3, Bc], BF, tag="tsig",
                                     name="tsig")
                      tg = Pt.tile([128, HK, Bc], BF, tag="tg", name="tg")
                      tcn = Pt.tile([128, HK, Bc], BF, tag="tcn", name="tcn")
                      if lv == 0:
                          # h0 == 0: gates are just gz; c0 == 0
                          nc.scalar.activation(tsig[:], t_gz[:, :, 0:3, cs],
                                               AF.Sigmoid)
                          nc.scalar.activation(tg[:], t_gz[:, :, 3, cs],
                                               AF.Tanh)
                          nc.vector.tensor_mul(t_cc[:], tsig[:, :, 0, :],
                                               tg[:])
                          nc.scalar.activation(tcn[:], t_cc[:], AF.Tanh)
                          nc.vector.tensor_mul(t_emb[:, :, cs],
                                               tsig[:, :, 2, :], tcn[:])
                      else:
                          ps = PSc.tile([128, HK, 4, Bc], F32, tag="cgp",
                                        name="cgp")
                          for p in range(HK):
                              for g in range(4):
                                  ms = slice((p * 4 + g) * 128,
                                             (p * 4 + g + 1) * 128)
                                  for k in range(0, HK, 2):
                                      nc.tensor.matmul(
                                          ps[:, p, g, :],
                                          t_cwhh[:, k:k + 2, ms],
                                          t_emb[:, k:k + 2, ps_prev],
                                          start=(k == 0),
                                          stop=(k == HK - 2), perf_mode=DR)
                          # previous level's ge fills the PE while this
                          # level's elementwise chain runs
                          ge_fill(lv - 1)
                          gs = Pt.tile([128, HK, 4, Bc], BF, tag="gs",
                                       name="gs")
                          tm1 = Pt.tile([128, HK, Bc], BF, tag="tm1",
                                        name="tm1")
                          tm2 = Pt.tile([128, HK, Bc], F32, tag="tm2",
                                        name="tm2")
                          # elementwise in two p-halves, pipelined across
                          # engines; emb half 0 lands early so the next
                          # level's k-outer matmuls can begin
                          for hp in (slice(0, HK // 2), slice(HK // 2, HK)):
                              nc.vector.tensor_add(gs[:, hp, :, :],
                                                   ps[:, hp, :, :],
                                                   t_gz[:, hp, :, cs])
                              nc.scalar.activation(tsig[:, hp, :, :],
                                                   gs[:, hp, 0:3, :],
                                                   AF.Sigmoid)
                              nc.scalar.activation(tg[:, hp, :],
                                                   gs[:, hp, 3, :], AF.Tanh)
                              nc.vector.tensor_mul(tm1[:, hp, :],
                                                   tsig[:, hp, 0, :],
                                                   tg[:, hp, :])
                              nc.gpsimd.tensor_mul(tm2[:, hp, :],
                                                   tsig[:, hp, 1, :],
                                                   t_cc[:, hp, :])
                              nc.vector.tensor_add(t_cc[:, hp, :],
                                                   tm1[:, hp, :],
                                                   tm2[:, hp, :])
                              nc.scalar.activation(tcn[:, hp, :],
                                                   t_cc[:, hp, :], AF.Tanh)
                              nc.vector.tensor_mul(t_emb[:, hp, cs],
                                                   tsig[:, hp, 2, :],
                                                   tcn[:, hp, :])
                  ge_fill(L - 1)

        # remaining decoder weights (first used at dec00 outproj / dec01)
        Pw2 = ctx.enter_context(tc.tile_pool(name="wdec2", bufs=1))
        t_dwn = Pw2.tile([128, TK, 4 * H], F8, tag="dwn")
        nc.sync.dma_start(t_dwn[:], d["dwn"][:])
        t_owt = Pw2.tile([128, HK, T], F8, tag="owt")
        nc.sync.dma_start(t_owt[:], d["owt"][:])

        # ---------------- decoder: 16 note steps over 512 rows --------------
        with tc.tile_pool(name="dtmp", bufs=4) as Pdt, \
             tc.tile_pool(name="dps", bufs=3, space="PSUM") as PSd, \
             tc.tile_pool(name="dpso", bufs=2, space="PSUM") as PSo:
            prefetched = {}
            for _drep in range(DEC_REPS):
              for t in range(NS):
                  _mark(nc, f"dec{t:02d}")
                  hin = t_h[t % 2]
                  hout = t_h[(t + 1) % 2]
                  psAB = {}
                  # software-pipelined emission: stage ops of chunk p are
                  # emitted after stage ops of chunk p+1's predecessors so
                  # each engine's FIFO never head-of-line blocks on a
                  # dependency that a later-emitted independent op could fill.
                  tiles = {}

                  def mms(p):
                      psA = PSd.tile([128, 2, R], F32, tag="dgp", name="psA")
                      if p in prefetched:
                          psB = prefetched.pop(p)
                      else:
                          psB = PSd.tile([128, 2, R], F32, tag="dgp",
                                         name="psB")
                          # g gate: emb contribution recomputed on PE
                          for k in range(0, HK, 2):
                              _mm_dr(nc, psB[:, 1, :], t_dweg, t_emb,
                                     k, slice(p * 128, (p + 1) * 128),
                                     (k == 0), False)
                      psAB[p] = (psA, psB)
                      for gi in range(4):
                          pst = psA if gi < 2 else psB
                          sl = gi % 2
                          ms = slice((p * 4 + gi) * 128,
                                     (p * 4 + gi + 1) * 128)
                          for k in range(0, HK, 2):
                              _mm_dr(nc, pst[:, sl, :], t_dwhh, hin, k, ms,
                                     (k == 0 and gi != 3),
                                     (t == 0 and k == HK - 2))
                          if t > 0:
                              for k in range(0, TK, 2):
                                  _mm_dr(nc, pst[:, sl, :], t_dwn, t_note8,
                                         k, ms, False, (k == TK - 2))

                  FULL = slice(0, R)
                  HALVES = (slice(0, R // 2), slice(R // 2, R))

                  def adds(p, cl=FULL, alloc=True):
                      psA, psB = psAB[p]
                      if alloc:
                          gs3 = Pdt.tile([128, 3, R], BF, tag="gs3",
                                         name="gs3")
                          tg = Pdt.tile([128, R], BF, tag="tg", name="tg")
                          tiles[p] = (gs3, tg)
                      gs3, tg = tiles[p]
                      nc.vector.tensor_add(gs3[:, 0:2, cl], psA[:, :, cl],
                                           t_ge[:, 3 * p:3 * p + 2, cl])
                      nc.vector.tensor_add(gs3[:, 2, cl], psB[:, 0, cl],
                                           t_ge[:, 3 * p + 2, cl])
                      nc.scalar.activation(tg[:, cl], psB[:, 1, cl], AF.Tanh,
                                           bias=t_dbg[:, p:p + 1])

                  def acts(p, cl=FULL, alloc=True):
                      if alloc:
                          gs3, tg = tiles[p]
                          tsig = Pdt.tile([128, 3, R], BF, tag="tsig3",
                                          name="tsig3")
                          tiles[p] = (tsig, tg, gs3)
                      tsig, tg, gs3 = tiles[p]
                      nc.scalar.activation(tsig[:, :, cl], gs3[:, :, cl],
                                           AF.Sigmoid)

                  def tail(p, cl=FULL, alloc=True):
                      tsig, tg = tiles[p][0], tiles[p][1]
                      if alloc:
                          tcn = Pdt.tile([128, R], BF, tag="tcn", name="tcn")
                          tm1 = Pdt.tile([128, R], BF, tag="tm1", name="tm1")
                          tm2 = Pdt.tile([128, R], F32, tag="tm2", name="tm2")
                          tiles[(p, 'x')] = (tcn, tm1, tm2)
                      tcn, tm1, tm2 = tiles[(p, 'x')]
                      nc.vector.tensor_mul(tm1[:, cl], tsig[:, 0, cl],
                                           tg[:, cl])
                      nc.gpsimd.tensor_mul(tm2[:, cl], tsig[:, 1, cl],
                                           t_c[:, p, cl])
                      nc.gpsimd.tensor_add(t_c[:, p, cl], tm1[:, cl],
                                           tm2[:, cl])
                      nc.scalar.activation(tcn[:, cl], t_c[:, p, cl], AF.Tanh)
                      nc.vector.tensor_mul(hout[:, p, cl], tsig[:, 2, cl],
                                           tcn[:, cl])

                  for p in range(HK + 3):
                      if p < HK:
                          mms(p)
                          if p < HK - 2:
                              adds(p)
                          else:
                              # last two chunks: half-R ops so the
                              # step-boundary chain pipelines at finer grain
                              adds(p, HALVES[0])
                              adds(p, HALVES[1], alloc=False)
                      if 1 <= p and p - 1 < HK:
                          if p - 1 < HK - 2:
                              acts(p - 1)
                          else:
                              acts(p - 1, HALVES[0])
                              acts(p - 1, HALVES[1], alloc=False)
                      if p >= 3:
                          q = p - 3
                          if q < HK - 2:
                              tail(q)
                          else:
                              tail(q, HALVES[0])
                              tail(q, HALVES[1], alloc=False)
                  # prefetch next step's g-gate emb matmuls ahead of the
                  # outproj in the PE FIFO: they are ready immediately and
                  # fill the tail wait, keeping the PE p-state warm
                  if t + 1 < NS:
                      for p in range(0):
                          psB = PSd.tile([128, 2, R], F32, tag="dgp",
                                         name="psBpre")
                          for k in range(0, HK, 2):
                              _mm_dr(nc, psB[:, 1, :], t_dweg, t_emb, k,
                                     slice(p * 128, (p + 1) * 128),
                                     (k == 0), False)
                          prefetched[p] = psB
                  # output projection + sigmoid -> note (fp8 mirror first so
                  # the next step's Wn matmuls unblock as early as possible)
                  for tk in range(TK):
                      ts_ = slice(tk * 128, (tk + 1) * 128)
                      po = PSo.tile([128, R], F32, tag="dpo", name="dpo")
                      for k in range(0, HK, 2):
                          _mm_dr(nc, po[:], t_owt, hout, k, ts_,
                                 (k == 0), (k == HK - 2))
                      nc.scalar.activation(t_note8[:, tk, :], po[:],
                                           AF.Sigmoid, bias=t_ob[:, tk:tk + 1])
                      nc.scalar.activation(t_note[:, tk, :], po[:],
                                           AF.Sigmoid, bias=t_ob[:, tk:tk + 1])
                      nc.sync.dma_start(d["outbuf"][t, tk], t_note[:, tk, :])


import os
DEC_REPS = int(os.environ.get("KBENCH_DEC_REPS", "1"))
COND_REPS = int(os.environ.get("KBENCH_COND_REPS", "1"))

_CACHE = {}


def _build():
    if "nc" not in _CACHE:
        nc = bacc.Bacc("TRN2", target_bir_lowering=False, debug=False,
                       num_devices=NCORES)
        d = _declare(nc)
        with tile.TileContext(nc) as tc:
            _body(nc, tc, d)
        nc.compile()
        _CACHE["nc"] = nc
    return _CACHE["nc"]


def _feat_major(W, dt):
    """[J, K] -> [128, K/128, J] (stationary lhsT chunk layout)."""
    J, K = W.shape
    return np.ascontiguousarray(
        W.reshape(J, K // 128, 128).transpose(2, 1, 0)).astype(dt)


def _reorder4(W):
    """[4H, K] rows in PyTorch gate blocks (i,f,g,o) -> p-adjacent chunks in
    order (i,f,o,g): new chunk m = p*4 + {0:i,1:f,2:o,3:g}."""
    K = W.shape[1]
    W4 = W.reshape(4, HK, 128, K)[[0, 1, 3, 2]]
    return np.ascontiguousarray(W4.transpose(1, 0, 2, 3).reshape(4 * H, K))


def _pack_inputs(inputs):
    z = np.asarray(inputs["z"], np.float32)
    dec_h0 = np.asarray(inputs["dec_h0"], np.float32)
    dec_c0 = np.asarray(inputs["dec_c0"], np.float32)
    cond_b = np.asarray(inputs["cond_bih"] + inputs["cond_bhh"], np.float32)
    dec_b = np.asarray(inputs["dec_bih"] + inputs["dec_bhh"], np.float32)
    out_b = np.asarray(inputs["out_b"], np.float32)

    cb4 = cond_b.reshape(4, HK, 128)[[0, 1, 3, 2]]
    cbt = np.ascontiguousarray(cb4.transpose(1, 0, 2).reshape(G, 128).T)
    db4 = dec_b.reshape(4, HK, 128)
    gebb = np.ascontiguousarray(
        db4[[0, 1, 3]].transpose(1, 0, 2).reshape(1, 3 * H))
    dbg = np.ascontiguousarray(db4[2].T)

    We = np.asarray(inputs["dec_Wih"][:, :H], np.float32)
    We4 = We.reshape(4, HK, 128, H)
    We3 = np.ascontiguousarray(
        We4[[0, 1, 3]].transpose(1, 0, 2, 3).reshape(3 * H, H))
    dwe = _feat_major(We3, f8)                      # [128, HK, 3H]
    dweg = _feat_major(np.ascontiguousarray(We4[2].reshape(H, H)), f8)

    shared = {
        "cbt": cbt.astype(np.float32),
        "dbg": dbg.astype(np.float32),
        "ones": np.ones((1, Bc), dtype=bf16),
        "gebb": gebb.astype(bf16),
        "obias": np.ascontiguousarray(out_b.reshape(TK, 128).T).astype(np.float32),
        "cwih": _feat_major(_reorder4(np.asarray(inputs["cond_Wih"], np.float32)), f8),
        "cwhh": _feat_major(_reorder4(np.asarray(inputs["cond_Whh"], np.float32)), f8),
        "dwn": _feat_major(_reorder4(np.asarray(inputs["dec_Wih"][:, H:], np.float32)), f8),
        "dwhh": _feat_major(_reorder4(np.asarray(inputs["dec_Whh"], np.float32)), f8),
        "owt": _feat_major(np.asarray(inputs["out_W"], np.float32), f8),
        "dwe": dwe,
        "dweg": dweg,
    }

    z_lv = z[:, np.arange(L) * L, 0, :]           # [B, L, Z]
    in_maps = []
    for c in range(NCORES):
        bs = slice(c * Bc, (c + 1) * Bc)
        zc = z_lv[bs]                              # [Bc, L, Z]
        zT = np.ascontiguousarray(
            zc.reshape(Bc, L, ZK, 128).transpose(3, 2, 1, 0).reshape(128, ZK, R)
        ).astype(f8)
        h0 = dec_h0[:, bs, :]                      # [L, Bc, H]
        h0T = np.ascontiguousarray(
            h0.reshape(L, Bc, HK, 128).transpose(3, 2, 0, 1).reshape(128, HK, R))
        c0 = dec_c0[:, bs, :]
        c0T = np.ascontiguousarray(
            c0.reshape(L, Bc, HK, 128).transpose(3, 2, 0, 1).reshape(128, HK, R))
        m = dict(shared)
        m["zT"] = zT
        m["h0T"] = h0T.astype(f8)
        m["c0T"] = c0T.astype(np.float32)
        in_maps.append(m)
    return in_maps


def _unpack_outputs(core_outs):
    notes = np.empty((B, L * NS, T), np.float32)
    for c, arr in enumerate(core_outs):
        # arr [NS, TK, 128, R] -> [Bc, L, NS, T]
        a = arr.astype(np.float32).reshape(NS, TK, 128, L, Bc).transpose(4, 3, 0, 1, 2)
        notes[c * Bc:(c + 1) * Bc] = a.reshape(Bc, L, NS, T).reshape(
            Bc, L * NS, T)
    return notes


def kernel(**inputs):
    nc = _build()
    in_maps = _pack_inputs(inputs)
    res = run_bass_kernel_spmd(nc, in_maps, list(range(NCORES)))
    return _unpack_outputs([r["outbuf"] for r in res.results])
